# revision 15
# baseline (speedup 1.0000x reference)
"""Trainium2 Bass kernel for nn_CosmosPatcher3d.

Computes the Cosmos 3D Haar wavelet patcher: input [1,3,33,704,704] fp32,
temporal causal pad (first frame repeated 4x -> 36 frames), then two full
3D Haar DWT levels. Equivalent to a separable +-1 Hadamard transform over
4x4x4 blocks scaled by 1/64, producing [1,192,9,176,176] fp32 with channel
layout ch = 3*sub + c, sub = 32*T2+16*H2+8*W2+4*T1+2*H1+W1.

Strategy (8 NeuronCores, shard along H: 704 = 8*88):
- The ENTIRE 4x4x4 transform is one TensorE matmul per output frame:
  contraction K = (dt(4), hh(8), r(4)) = 128 covers a full temporal block,
  8 input rows (2 output rows y'), and the W-position within a 4-block;
  M = (y'(2), sub(64)) = 128 gives all 64 subbands for both rows.
- The host pre-packs the input into the exact SBUF tile image
  [1056, 5808] bf16 (rows = (t, K), cols = (c, g, x')), so every load is
  one fully contiguous [128-row, 11.6KB-per-row] transfer that sprays
  across all 16 SDMA engines (the outermost AP dim is what gets sprayed).
- Output is stored as the raw tile image [8, 128, 5808] bf16 for t>=1
  (absmax rel err ~4e-3, gate 2e-2); host casts to f32 and unpermutes.
- t=0 (first-frame repeat) has only 16 nonzero subbands (T2=T1=0), so it
  uses a compact M=32 matmul and stores only [32, 5808]; host scatters.
- Queue layout: loads on the Sync HWDGE ring (all prefetched up front),
  stores on the GpSimd SWDGE ring, PSUM->SBUF copies split across
  Scalar and Vector. Three independent issue queues keep all 16 SDMA
  engines saturated with zero cross-blocking.
"""

import ml_dtypes
import numpy as np

import concourse.bacc as bacc
import concourse.mybir as mybir
import concourse.tile as tile
from concourse.bass_utils import run_bass_kernel_spmd

N_CORES = 8
C = 3            # input channels
T_IN = 33        # input frames
H_IN = 704       # input height (global)
W_IN = 704       # input width
H_SH = H_IN // N_CORES      # 88 input rows per core
T_OUT = 9
G = H_SH // 8               # 11 row-pair groups per core
X_OUT = W_IN // 4           # 176
CFREE = G * X_OUT           # 1936 free elements per channel
FREE = C * CFREE            # 5808
ROWS = 32 + 8 * 128         # 1056 packed input rows per core

_F32 = mybir.dt.float32
_BF16 = mybir.dt.bfloat16
_BF16_NP = ml_dtypes.bfloat16


def _sgn(p, b2, b1):
    """Composite 2-level Haar sign for position p in 0..3 (+-1)."""
    s = 1.0
    if b1:
        s *= 1.0 - 2.0 * (p % 2)
    if b2:
        s *= 1.0 - 2.0 * (p // 2)
    return s


def _build_lhst():
    """Sign matrices incl. the global 1/64 scale (exact in bf16).

    l1 [K=128, M=128]: k = dt*32 + hh*4 + r; m = y'*64 + sub.
    l0 [K=32, M=32]:   k = hh*4 + r (frame 0 repeated 4x -> weight 4);
    m0 = y'*16 + (h2*8 + w2*4 + h1*2 + w1), only T2=T1=0 subbands.
    """
    l1 = np.zeros((128, 128), dtype=np.float32)
    l0 = np.zeros((32, 32), dtype=np.float32)
    for m in range(128):
        yp, sub = m // 64, m % 64
        t2, h2, w2 = (sub >> 5) & 1, (sub >> 4) & 1, (sub >> 3) & 1
        t1, h1, w1 = (sub >> 2) & 1, (sub >> 1) & 1, sub & 1
        for hh in range(8):
            if hh // 4 != yp:
                continue
            sh = _sgn(hh % 4, h2, h1)
            for r in range(4):
                sw = _sgn(r, w2, w1)
                for dt in range(4):
                    st = _sgn(dt, t2, t1)
                    l1[dt * 32 + hh * 4 + r, m] = st * sh * sw / 64.0
                if t2 == 0 and t1 == 0:
                    m0 = yp * 16 + h2 * 8 + w2 * 4 + h1 * 2 + w1
                    l0[hh * 4 + r, m0] = 4.0 * sh * sw / 64.0
    return l1.astype(_BF16_NP), l0.astype(_BF16_NP)


def _build_nc():
    nc = bacc.Bacc(
        "TRN2", target_bir_lowering=False, debug=False, num_devices=N_CORES
    )
    x = nc.dram_tensor("x", [ROWS, FREE], _BF16, kind="ExternalInput").ap()
    l1 = nc.dram_tensor("l1", [128, 128], _BF16, kind="ExternalInput").ap()
    l0 = nc.dram_tensor("l0", [32, 32], _BF16, kind="ExternalInput").ap()
    out = nc.dram_tensor(
        "out", [T_OUT - 1, 128, FREE], _BF16, kind="ExternalOutput"
    ).ap()
    out0 = nc.dram_tensor(
        "out0", [32, FREE], _BF16, kind="ExternalOutput"
    ).ap()

    # out viewed per (t, c) for per-channel stores
    o_v = out.rearrange("t m (c f) -> t m c f", c=C)

    HALF = CFREE // 2  # 968 = 512 + 456, 2 PSUM banks

    with tile.TileContext(nc) as tc:
        with (
            tc.tile_pool(name="signs", bufs=1) as sgp,
            tc.tile_pool(name="rhs", bufs=T_OUT) as rhp,
            tc.tile_pool(name="outp", bufs=3) as otp,
            tc.tile_pool(name="psum", bufs=4, space="PSUM") as psp,
        ):
            tl1 = sgp.tile([128, 128], _BF16)
            tl0 = sgp.tile([32, 32], _BF16)
            # prefetch everything up front: t=0 deps first, then all t
            nc.sync.dma_start(out=tl0, in_=l0)
            rhss = []
            for t in range(T_OUT):
                kdim = 32 if t == 0 else 128
                row0 = 0 if t == 0 else 32 + 128 * (t - 1)
                rhs = rhp.tile([128, C, CFREE], _BF16, tag="rhs")
                nc.sync.dma_start(
                    out=rhs[:kdim], in_=x[row0 : row0 + kdim]
                )
                rhss.append(rhs)
                if t == 0:
                    nc.sync.dma_start(out=tl1, in_=l1)

            eng_i = 0

            def do_frame(lhsT, rhs, kdim, mdim, ot):
                nonlocal eng_i
                for c in range(C):
                    for h in range(2):
                        j0 = h * HALF
                        ps = psp.tile([mdim, HALF], _F32, tag="ps")
                        for j in (0, 512):
                            n = min(512, HALF - j)
                            nc.tensor.matmul(
                                ps[:, j : j + n],
                                lhsT,
                                rhs[:kdim, c, j0 + j : j0 + j + n],
                                start=True,
                                stop=True,
                            )
                        # alternate PSUM->SBUF copies across Scalar/Vector
                        dst = ot[:mdim, c, j0 : j0 + HALF]
                        if eng_i % 2 == 0:
                            nc.scalar.copy(out=dst, in_=ps)
                        else:
                            nc.vector.tensor_copy(dst, ps)
                        eng_i += 1

            # t=0: compact M=32 (only T2=T1=0 subbands), single store
            ot0 = otp.tile([32, C, CFREE], _BF16, tag="ot0", bufs=1)
            do_frame(tl0, rhss[0], 32, 32, ot0)
            nc.gpsimd.dma_start(out=out0, in_=ot0)

            for t in range(1, T_OUT):
                ot = otp.tile([128, C, CFREE], _BF16, tag="ot")
                do_frame(tl1, rhss[t], 128, 128, ot)
                for c in range(C):
                    # stores ride the GpSimd SWDGE ring (own queue)
                    nc.gpsimd.dma_start(out=o_v[t - 1, :, c], in_=ot[:, c])

    nc.compile()
    return nc


_NC_CACHE = None


def _prep_inputs(hs):
    """Shard along H and pack each core's input into the SBUF tile image.

    Row layout: rows 0..31 are t=0 (k = hh*4 + r over frame 0); rows
    32+128*(t-1)+k for t>=1 with k = dt*32 + hh*4 + r reading frame
    4t-3+dt. Column layout: c*1936 + g*176 + x' with w = 4*x' + r,
    h = 8*g + hh.
    """
    l1, l0 = _build_lhst()
    hsv = hs[0]  # [C, T, H, W]
    in_maps = []
    for k in range(N_CORES):
        xk = np.ascontiguousarray(
            hsv[:, :, k * H_SH : (k + 1) * H_SH, :]
        ).astype(_BF16_NP)
        xv = xk.reshape(C, T_IN, G, 8, X_OUT, 4)  # c, T, g, hh, x', r
        # t=0: [hh, r, c, g, x']
        t0 = np.ascontiguousarray(
            xv[:, 0].transpose(2, 4, 0, 1, 3)
        ).reshape(32, FREE)
        # t>=1: [t8, dt, hh, r, c, g, x']
        xt = xv[:, 1:].reshape(C, 8, 4, G, 8, X_OUT, 4)
        xt = np.ascontiguousarray(
            xt.transpose(1, 2, 4, 6, 0, 3, 5)
        ).reshape(8 * 128, FREE)
        xr = np.concatenate([t0, xt], axis=0)
        in_maps.append({"x": xr, "l1": l1, "l0": l0})
    return in_maps


# scatter map: compact t=0 row (h2,w2,h1,w1) -> full sub index
_SUB0 = np.array(
    [
        ((m >> 3) & 1) * 16 + ((m >> 2) & 1) * 8 + ((m >> 1) & 1) * 2 + (m & 1)
        for m in range(16)
    ]
)


def kernel(hidden_states: np.ndarray) -> np.ndarray:
    global _NC_CACHE
    if _NC_CACHE is None:
        _NC_CACHE = _build_nc()
    nc = _NC_CACHE

    hs = np.asarray(hidden_states, dtype=np.float32)
    assert hs.shape == (1, C, T_IN, H_IN, W_IN), hs.shape
    in_maps = _prep_inputs(hs)

    res = run_bass_kernel_spmd(nc, in_maps, core_ids=list(range(N_CORES)))

    out = np.empty((1, 192, T_OUT, H_IN // 4, X_OUT), dtype=np.float32)
    y_sh = H_SH // 4  # 22
    for k in range(N_CORES):
        full = np.zeros((T_OUT, 2, 64, FREE), dtype=np.float32)
        full[1:] = (
            res.results[k]["out"].astype(np.float32).reshape(8, 2, 64, FREE)
        )
        o0 = res.results[k]["out0"].astype(np.float32).reshape(2, 16, FREE)
        full[0][:, _SUB0] = o0
        r = full.reshape(T_OUT, 2, 64, C, G, X_OUT)  # t, y', sub, c, g, x'
        r = r.transpose(2, 3, 0, 4, 1, 5).reshape(192, T_OUT, y_sh, X_OUT)
        out[0, :, :, k * y_sh : (k + 1) * y_sh, :] = r
    return out


# revision 16
# speedup vs baseline: 1.0350x; 1.0350x over previous
"""Trainium2 Bass kernel for nn_CosmosPatcher3d.

Computes the Cosmos 3D Haar wavelet patcher: input [1,3,33,704,704] fp32,
temporal causal pad (first frame repeated 4x -> 36 frames), then two full
3D Haar DWT levels. Equivalent to a separable +-1 Hadamard transform over
4x4x4 blocks scaled by 1/64, producing [1,192,9,176,176] fp32 with channel
layout ch = 3*sub + c, sub = 32*T2+16*H2+8*W2+4*T1+2*H1+W1.

Strategy (8 NeuronCores, shard along H: 704 = 8*88):
- The ENTIRE 4x4x4 transform is one TensorE matmul per output frame:
  contraction K = (dt(4), hh(8), r(4)) = 128 covers a full temporal block,
  8 input rows (2 output rows y'), and the W-position within a 4-block;
  M = (y'(2), sub(64)) = 128 gives all 64 subbands for both rows.
- The host pre-packs the input into the exact SBUF tile image
  [1056, 5808] bf16 (rows = (t, K), cols = (c, g, x')), so every load is
  one fully contiguous [128-row, 11.6KB-per-row] transfer that sprays
  across all 16 SDMA engines (the outermost AP dim is what gets sprayed).
- Output is stored as the raw tile image [8, 128, 5808] bf16 for t>=1
  (absmax rel err ~4e-3, gate 2e-2); host casts to f32 and unpermutes.
- t=0 (first-frame repeat) has only 16 nonzero subbands (T2=T1=0), so it
  uses a compact M=32 matmul and stores only [32, 5808]; host scatters.
- Queue layout: loads on the Sync HWDGE ring (all prefetched up front),
  stores on the GpSimd SWDGE ring, PSUM->SBUF copies split across
  Scalar and Vector. Three independent issue queues keep all 16 SDMA
  engines saturated with zero cross-blocking.
"""

import ml_dtypes
import numpy as np

import concourse.bacc as bacc
import concourse.mybir as mybir
import concourse.tile as tile
from concourse.bass_utils import run_bass_kernel_spmd

N_CORES = 8
C = 3            # input channels
T_IN = 33        # input frames
H_IN = 704       # input height (global)
W_IN = 704       # input width
H_SH = H_IN // N_CORES      # 88 input rows per core
T_OUT = 9
G = H_SH // 8               # 11 row-pair groups per core
X_OUT = W_IN // 4           # 176
CFREE = G * X_OUT           # 1936 free elements per channel
FREE = C * CFREE            # 5808
ROWS = 32 + 8 * 128         # 1056 packed input rows per core

_F32 = mybir.dt.float32
_BF16 = mybir.dt.bfloat16
_BF16_NP = ml_dtypes.bfloat16


def _sgn(p, b2, b1):
    """Composite 2-level Haar sign for position p in 0..3 (+-1)."""
    s = 1.0
    if b1:
        s *= 1.0 - 2.0 * (p % 2)
    if b2:
        s *= 1.0 - 2.0 * (p // 2)
    return s


def _build_lhst():
    """Sign matrices incl. the global 1/64 scale (exact in bf16).

    l1 [K=128, M=128]: k = dt*32 + hh*4 + r; m = y'*64 + sub.
    l0 [K=32, M=32]:   k = hh*4 + r (frame 0 repeated 4x -> weight 4);
    m0 = y'*16 + (h2*8 + w2*4 + h1*2 + w1), only T2=T1=0 subbands.
    """
    l1 = np.zeros((128, 128), dtype=np.float32)
    l0 = np.zeros((32, 32), dtype=np.float32)
    for m in range(128):
        yp, sub = m // 64, m % 64
        t2, h2, w2 = (sub >> 5) & 1, (sub >> 4) & 1, (sub >> 3) & 1
        t1, h1, w1 = (sub >> 2) & 1, (sub >> 1) & 1, sub & 1
        for hh in range(8):
            if hh // 4 != yp:
                continue
            sh = _sgn(hh % 4, h2, h1)
            for r in range(4):
                sw = _sgn(r, w2, w1)
                for dt in range(4):
                    st = _sgn(dt, t2, t1)
                    l1[dt * 32 + hh * 4 + r, m] = st * sh * sw / 64.0
                if t2 == 0 and t1 == 0:
                    m0 = yp * 16 + h2 * 8 + w2 * 4 + h1 * 2 + w1
                    l0[hh * 4 + r, m0] = 4.0 * sh * sw / 64.0
    return l1.astype(_BF16_NP), l0.astype(_BF16_NP)


def _build_nc():
    nc = bacc.Bacc(
        "TRN2", target_bir_lowering=False, debug=False, num_devices=N_CORES
    )
    x = nc.dram_tensor("x", [ROWS, FREE], _BF16, kind="ExternalInput").ap()
    l1 = nc.dram_tensor("l1", [128, 128], _BF16, kind="ExternalInput").ap()
    l0 = nc.dram_tensor("l0", [32, 32], _BF16, kind="ExternalInput").ap()
    out = nc.dram_tensor(
        "out", [T_OUT - 1, 128, FREE], _BF16, kind="ExternalOutput"
    ).ap()
    out0 = nc.dram_tensor(
        "out0", [32, FREE], _BF16, kind="ExternalOutput"
    ).ap()

    # out viewed per (t, c) for per-channel stores
    o_v = out.rearrange("t m (c f) -> t m c f", c=C)

    HALF = CFREE // 2  # 968 = 512 + 456, 2 PSUM banks

    with tile.TileContext(nc) as tc:
        with (
            tc.tile_pool(name="signs", bufs=1) as sgp,
            tc.tile_pool(name="rhs", bufs=T_OUT) as rhp,
            tc.tile_pool(name="outp", bufs=3) as otp,
            tc.tile_pool(name="psum", bufs=4, space="PSUM") as psp,
        ):
            tl1 = sgp.tile([128, 128], _BF16)
            tl0 = sgp.tile([32, 32], _BF16)
            # lhsT loads ride the Scalar ring so the Sync ring streams
            # rhs data from the first instant; rhs loads alternate
            # between the two HWDGE rings to feed the SDMA engines from
            # two descriptor queues concurrently.
            nc.scalar.dma_start(out=tl0, in_=l0)
            nc.scalar.dma_start(out=tl1, in_=l1)
            rhss = []
            for t in range(T_OUT):
                kdim = 32 if t == 0 else 128
                row0 = 0 if t == 0 else 32 + 128 * (t - 1)
                rhs = rhp.tile([128, C, CFREE], _BF16, tag="rhs")
                eng = nc.sync if t % 2 == 0 else nc.scalar
                eng.dma_start(out=rhs[:kdim], in_=x[row0 : row0 + kdim])
                rhss.append(rhs)

            eng_i = 0

            def do_frame(lhsT, rhs, kdim, mdim, ot):
                nonlocal eng_i
                for c in range(C):
                    for h in range(2):
                        j0 = h * HALF
                        ps = psp.tile([mdim, HALF], _F32, tag="ps")
                        for j in (0, 512):
                            n = min(512, HALF - j)
                            nc.tensor.matmul(
                                ps[:, j : j + n],
                                lhsT,
                                rhs[:kdim, c, j0 + j : j0 + j + n],
                                start=True,
                                stop=True,
                            )
                        # alternate PSUM->SBUF copies across Scalar/Vector
                        dst = ot[:mdim, c, j0 : j0 + HALF]
                        if eng_i % 2 == 0:
                            nc.scalar.copy(out=dst, in_=ps)
                        else:
                            nc.vector.tensor_copy(dst, ps)
                        eng_i += 1

            # t=0: compact M=32 (only T2=T1=0 subbands), single store
            ot0 = otp.tile([32, C, CFREE], _BF16, tag="ot0", bufs=1)
            do_frame(tl0, rhss[0], 32, 32, ot0)
            nc.gpsimd.dma_start(out=out0, in_=ot0)

            for t in range(1, T_OUT):
                ot = otp.tile([128, C, CFREE], _BF16, tag="ot")
                do_frame(tl1, rhss[t], 128, 128, ot)
                for c in range(C):
                    # stores ride the GpSimd SWDGE ring (own queue)
                    nc.gpsimd.dma_start(out=o_v[t - 1, :, c], in_=ot[:, c])

    nc.compile()
    return nc


_NC_CACHE = None


def _prep_inputs(hs):
    """Shard along H and pack each core's input into the SBUF tile image.

    Row layout: rows 0..31 are t=0 (k = hh*4 + r over frame 0); rows
    32+128*(t-1)+k for t>=1 with k = dt*32 + hh*4 + r reading frame
    4t-3+dt. Column layout: c*1936 + g*176 + x' with w = 4*x' + r,
    h = 8*g + hh.
    """
    l1, l0 = _build_lhst()
    hsv = hs[0]  # [C, T, H, W]
    in_maps = []
    for k in range(N_CORES):
        xk = np.ascontiguousarray(
            hsv[:, :, k * H_SH : (k + 1) * H_SH, :]
        ).astype(_BF16_NP)
        xv = xk.reshape(C, T_IN, G, 8, X_OUT, 4)  # c, T, g, hh, x', r
        # t=0: [hh, r, c, g, x']
        t0 = np.ascontiguousarray(
            xv[:, 0].transpose(2, 4, 0, 1, 3)
        ).reshape(32, FREE)
        # t>=1: [t8, dt, hh, r, c, g, x']
        xt = xv[:, 1:].reshape(C, 8, 4, G, 8, X_OUT, 4)
        xt = np.ascontiguousarray(
            xt.transpose(1, 2, 4, 6, 0, 3, 5)
        ).reshape(8 * 128, FREE)
        xr = np.concatenate([t0, xt], axis=0)
        in_maps.append({"x": xr, "l1": l1, "l0": l0})
    return in_maps


# scatter map: compact t=0 row (h2,w2,h1,w1) -> full sub index
_SUB0 = np.array(
    [
        ((m >> 3) & 1) * 16 + ((m >> 2) & 1) * 8 + ((m >> 1) & 1) * 2 + (m & 1)
        for m in range(16)
    ]
)


def kernel(hidden_states: np.ndarray) -> np.ndarray:
    global _NC_CACHE
    if _NC_CACHE is None:
        _NC_CACHE = _build_nc()
    nc = _NC_CACHE

    hs = np.asarray(hidden_states, dtype=np.float32)
    assert hs.shape == (1, C, T_IN, H_IN, W_IN), hs.shape
    in_maps = _prep_inputs(hs)

    res = run_bass_kernel_spmd(nc, in_maps, core_ids=list(range(N_CORES)))

    out = np.empty((1, 192, T_OUT, H_IN // 4, X_OUT), dtype=np.float32)
    y_sh = H_SH // 4  # 22
    for k in range(N_CORES):
        full = np.zeros((T_OUT, 2, 64, FREE), dtype=np.float32)
        full[1:] = (
            res.results[k]["out"].astype(np.float32).reshape(8, 2, 64, FREE)
        )
        o0 = res.results[k]["out0"].astype(np.float32).reshape(2, 16, FREE)
        full[0][:, _SUB0] = o0
        r = full.reshape(T_OUT, 2, 64, C, G, X_OUT)  # t, y', sub, c, g, x'
        r = r.transpose(2, 3, 0, 4, 1, 5).reshape(192, T_OUT, y_sh, X_OUT)
        out[0, :, :, k * y_sh : (k + 1) * y_sh, :] = r
    return out
